# revision 32
# baseline (speedup 1.0000x reference)
"""AttentionBlock (GroupNorm + 4-head self-attention + out-proj + residual)
as a Bass/Tile kernel for 8 Trainium2 NeuronCores.

Sharding: 8 cores = 4 batches x 2 head-pairs. Core c handles batch c//2 and
heads {2*(c%2), 2*(c%2)+1}. Each core computes GroupNorm + QKV for its batch
(duplicated across the 2 cores of a batch), attention for its 2 heads, and a
partial output projection. Host sums the two partials per batch and adds b_out.

Layout strategy: matmuls contract over partitions, so xn lives in transposed
[C, N] layout (f32r) for QKV; attention runs in S^T orientation ([j, i]) so
softmax row sums come from a ones-column appended to V in the PV matmul; the
output projection contracts over head columns taking O^T directly, landing in
natural [N, C] layout where the softmax division is a per-partition scalar.
SBUF slots are aliased via shared pool tags across kernel phases.
"""

import sys

sys.path.insert(0, "/opt/trn_rl_repo")

from contextlib import ExitStack

import numpy as np

import concourse.bass as bass
import concourse.bacc as bacc
import concourse.tile as tile
from concourse import mybir
from concourse import bass_utils
from concourse.masks import make_identity

F32 = mybir.dt.float32
F32R = mybir.dt.float32r
ALU = mybir.AluOpType
ACTF = mybir.ActivationFunctionType

B, H, W, C = 4, 64, 64, 256
N = H * W               # 4096
NB = N // 128           # 32 natural row-blocks
NH = 4                  # heads in reference
HD = C // NH            # 64 head dim
NG = 8                  # groupnorm groups
GS = C // NG            # 32 channels per group
EPS = 1e-5
SCALE = C ** -0.5       # 1/16 attention scale
IC = 512                # attention i-chunk
NIC = N // IC           # 8
SG = 3                  # S^T psum banks per exp call

_CACHED = {}


def _build():
    nc = bacc.Bacc("TRN2", target_bir_lowering=False, debug=False, num_devices=8)

    x_d = nc.dram_tensor("x", [N, C], F32, kind="ExternalInput")
    wqkv_d = nc.dram_tensor("wqkv", [C, 384], F32, kind="ExternalInput")
    wout_d = nc.dram_tensor("wout", [128, C], F32, kind="ExternalInput")
    gns_d = nc.dram_tensor("gns", [C], F32, kind="ExternalInput")
    gnb_d = nc.dram_tensor("gnb", [C], F32, kind="ExternalInput")
    flag_d = nc.dram_tensor("flag", [128, 1], F32, kind="ExternalInput")
    out_d = nc.dram_tensor("out", [N, C], F32, kind="ExternalOutput")

    with tile.TileContext(nc) as tc:
        _body(tc, nc, x_d, wqkv_d, wout_d, gns_d, gnb_d, flag_d, out_d)
    nc.compile()
    return nc


def _body(tc, nc, x_d, wqkv_d, wout_d, gns_d, gnb_d, flag_d, out_d):
    v = nc.vector
    x_ap = x_d.ap().rearrange("(a b) c -> b a c", b=128)      # [128, 32, 256]
    out_ap = out_d.ap().rearrange("(a b) c -> b a c", b=128)

    with ExitStack() as stk:
        const = stk.enter_context(tc.tile_pool(name="const", bufs=1))
        big = stk.enter_context(tc.tile_pool(name="big", bufs=1))

        # ---- constants ----
        ident = const.tile([128, 128], F32)
        make_identity(nc, ident)
        ones1_128 = const.tile([1, 128], F32)
        nc.gpsimd.memset(ones1_128, 1.0)
        ones_f = const.tile([128, NB], F32)
        nc.gpsimd.memset(ones_f, 1.0)
        ind8 = []                 # per ct: [128, 8], partition q -> col 1 at (128ct+q)//32
        for ct in range(2):
            t = const.tile([128, 8], F32, tag=f"ind8_{ct}")
            nc.gpsimd.memset(t, 0.0)
            for g in range(4):
                nc.gpsimd.memset(t[32 * g:32 * g + 32, 4 * ct + g:4 * ct + g + 1], 1.0)
            ind8.append(t)
        indt = []                 # per ct: [8, 128] group -> channels of ct
        for ct in range(2):
            t = const.tile([8, 128], F32, tag=f"indt{ct}")
            nc.gpsimd.memset(t, 0.0)
            nc.gpsimd.affine_select(
                out=t.rearrange("p (a b) -> p a b", a=4), in_=t.rearrange("p (a b) -> p a b", a=4),
                compare_op=ALU.not_equal, fill=1.0, base=-4 * ct,
                pattern=[[-1, 4], [0, 32]], channel_multiplier=1)
            indt.append(t)
        ind_row = const.tile([8, 256], F32)   # group g -> channel row mask
        nc.gpsimd.memset(ind_row, 0.0)
        nc.gpsimd.affine_select(
            out=ind_row.rearrange("p (a b) -> p a b", a=8),
            in_=ind_row.rearrange("p (a b) -> p a b", a=8),
            compare_op=ALU.not_equal, fill=1.0, base=0,
            pattern=[[-1, 8], [0, 32]], channel_multiplier=1)

        gns_col = const.tile([128, 2], F32)
        nc.sync.dma_start(out=gns_col, in_=gns_d.ap().rearrange("(a b) -> b a", b=128))
        gnb_col = const.tile([128, 2], F32)
        nc.sync.dma_start(out=gnb_col, in_=gnb_d.ap().rearrange("(a b) -> b a", b=128))
        gns_row = const.tile([1, 256], F32)
        nc.sync.dma_start(out=gns_row, in_=gns_d.ap()[None, :])
        gnb_row = const.tile([1, 256], F32)
        nc.sync.dma_start(out=gnb_row, in_=gnb_d.ap()[None, :])
        flag_col = const.tile([128, 1], F32)
        nc.sync.dma_start(out=flag_col, in_=flag_d.ap())
        abf = const.tile([128, 512], F32)     # flag * (A | B) broadcast, set later
        wout_r = []
        for hl in range(2):
            wf = const.tile([64, 256], F32, tag=f"wf{hl}")
            nc.sync.dma_start(out=wf, in_=wout_d.ap()[64 * hl:64 * hl + 64, :])
            wr = const.tile([64, 256], F32R, tag=f"wr{hl}")
            v.tensor_copy(out=wr, in_=wf)
            wout_r.append(wr)
        wq_f = const.tile([128, 2, 384], F32)
        nc.sync.dma_start(out=wq_f, in_=wqkv_d.ap().rearrange("(a p) n -> p a n", p=128))
        wq_r = const.tile([128, 2, 384], F32R)
        v.tensor_copy(out=wq_r, in_=wq_f)

        # ---- persistent slots (aliased across phases via tags) ----
        x_nat = big.tile([128, NB, C], F32, tag="slotA")          # whole kernel
        for k in range(4):
            nc.sync.dma_start(out=x_nat[:, 8 * k:8 * k + 8, :],
                              in_=x_ap[:, 8 * k:8 * k + 8, :])

        # ---- transpose x -> xT [c, i] ----
        xt = big.tile([128, 2, N], F32, tag="slotB")              # -> ot_raw later
        with tc.tile_pool(name="ps_t", bufs=4, space="PSUM") as ps_t:
            for blk in range(NB):
                for ct in range(2):
                    p = ps_t.tile([128, 128], F32, tag="pt")
                    nc.tensor.transpose(p, x_nat[:, blk, 128 * ct:128 * ct + 128], ident)
                    v.tensor_copy(out=xt[:, ct, 128 * blk:128 * blk + 128], in_=p)

        # ---- groupnorm stats + normalize ----
        xnt = big.tile([128, 2, N], F32R, tag="slotD")            # -> out_sb later
        with tc.tile_pool(name="stats", bufs=1) as stp, \
             tc.tile_pool(name="ps_s", bufs=1, space="PSUM") as ps_s:
            abrow = stp.tile([1, 512], F32)
            a_col = stp.tile([128, 2], F32)
            b_col = stp.tile([128, 2], F32)
            gp8 = ps_s.tile([8, 2], F32)
            for ct in range(2):
                bst = stp.tile([128, 8, 6], F32, tag="bst")
                for s in range(8):
                    v.bn_stats(out=bst[:, s, :], in_=xt[:, ct, 512 * s:512 * s + 512])
                mv = stp.tile([128, 2], F32, tag="mv")
                v.bn_aggr(out=mv, in_=bst)
                mm = stp.tile([128, 1], F32, tag="mm")
                v.tensor_scalar(out=mm, in0=mv[:, 0:1], scalar1=mv[:, 0:1], scalar2=None,
                                op0=ALU.mult)
                m1m2 = stp.tile([128, 2], F32, tag="m1m2")
                v.tensor_copy(out=m1m2[:, 0:1], in_=mv[:, 0:1])
                v.tensor_scalar(out=m1m2[:, 1:2], in0=mv[:, 1:2], scalar1=mm, scalar2=None,
                                op0=ALU.add)
                nc.tensor.matmul(gp8, ind8[ct], m1m2, start=(ct == 0), stop=(ct == 1))
            # group stats (gp8 cols: [sum mean_c, sum m2_c]) -> mean_g, rstd_g [8,1]
            mg8 = stp.tile([8, 1], F32)
            v.tensor_scalar(out=mg8, in0=gp8[:, 0:1], scalar1=1.0 / GS, scalar2=None,
                            op0=ALU.mult)
            e2g = stp.tile([8, 1], F32)
            v.tensor_scalar(out=e2g, in0=gp8[:, 1:2], scalar1=1.0 / GS, scalar2=None,
                            op0=ALU.mult)
            varg = stp.tile([8, 1], F32)
            v.tensor_scalar(out=varg, in0=mg8, scalar1=mg8, scalar2=None, op0=ALU.mult)
            v.tensor_sub(out=varg, in0=e2g, in1=varg)
            epst = stp.tile([8, 1], F32)
            v.memset(epst, EPS)
            stdg = stp.tile([8, 1], F32)
            nc.scalar.activation(out=stdg, in_=varg, func=ACTF.Sqrt, bias=epst)
            rstdg = stp.tile([8, 1], F32)
            v.reciprocal(out=rstdg, in_=stdg)
            grp2 = stp.tile([8, 2], F32)
            v.tensor_copy(out=grp2[:, 0:1], in_=mg8)
            v.tensor_copy(out=grp2[:, 1:2], in_=rstdg)
            # per-channel A,B in column layout (for xnt)
            for ct in range(2):
                bc = ps_s.tile([128, 2], F32, tag="bc")
                nc.tensor.matmul(bc, indt[ct], grp2, start=True, stop=True)
                v.tensor_mul(out=a_col[:, ct:ct + 1], in0=bc[:, 1:2],
                             in1=gns_col[:, ct:ct + 1])
                mA = stp.tile([128, 1], F32, tag="mA")
                v.tensor_mul(out=mA, in0=bc[:, 0:1], in1=a_col[:, ct:ct + 1])
                v.tensor_sub(out=b_col[:, ct:ct + 1], in0=gnb_col[:, ct:ct + 1], in1=mA)
            # per-channel A,B row layout -> flag * broadcast [128, A|B]
            mr_m = ps_s.tile([1, 256], F32, tag="mr_m")
            nc.tensor.matmul(mr_m, mg8, ind_row, start=True, stop=True)
            mr_r = ps_s.tile([1, 256], F32, tag="mr_r")
            nc.tensor.matmul(mr_r, rstdg, ind_row, start=True, stop=True)
            v.tensor_mul(out=abrow[:, 0:256], in0=mr_r, in1=gns_row)
            marow = stp.tile([1, 256], F32)
            v.tensor_mul(out=marow, in0=mr_m, in1=abrow[:, 0:256])
            v.tensor_sub(out=abrow[:, 256:512], in0=gnb_row, in1=marow)
            abp = ps_s.tile([128, 512], F32, tag="abp")
            nc.tensor.matmul(abp, ones1_128, abrow, start=True, stop=True)
            v.tensor_scalar_mul(out=abf, in0=abp, scalar1=flag_col)

            # normalize transposed copy in place: xnt = xt * A + B (f32r)
            for ct in range(2):
                v.tensor_scalar(out=xnt[:, ct, :], in0=xt[:, ct, :],
                                scalar1=a_col[:, ct:ct + 1], scalar2=b_col[:, ct:ct + 1],
                                op0=ALU.mult, op1=ALU.add)

        # ---- QKV^T = wqkv_sh^T @ xn^T : q/k/v each [128, N] ----
        qt = big.tile([128, N], F32R, tag="slotQ")
        kt = big.tile([128, N], F32R, tag="slotK")
        vt = big.tile([128, N], F32, tag="slotC")
        with tc.tile_pool(name="ps_qkv", bufs=4, space="PSUM") as ps_q:
            for qi, dst in ((0, qt), (1, kt), (2, vt)):
                for ic in range(NIC):
                    p = ps_q.tile([128, 512], F32, tag="pq")
                    for ct in range(2):
                        nc.tensor.matmul(p, wq_r[:, ct, 128 * qi:128 * qi + 128],
                                         xnt[:, ct, IC * ic:IC * ic + IC],
                                         start=(ct == 0), stop=(ct == 1))
                    v.tensor_copy(out=dst[:, IC * ic:IC * ic + IC], in_=p)

        # ---- V^T -> V natural with ones columns ----
        v_ext = big.tile([128, NB, 130], F32R, tag="slotV")
        v.tensor_copy(out=v_ext[:, :, 64:65], in_=ones_f[:, :, None])
        v.tensor_copy(out=v_ext[:, :, 129:130], in_=ones_f[:, :, None])
        with tc.tile_pool(name="ps_vt", bufs=4, space="PSUM") as ps_v:
            for jb in range(NB):
                p = ps_v.tile([128, 128], F32, tag="pv")
                nc.tensor.transpose(p, vt[:, 128 * jb:128 * jb + 128], ident)
                v.tensor_copy(out=v_ext[:, jb, 0:64], in_=p[:, 0:64])
                v.tensor_copy(out=v_ext[:, jb, 65:129], in_=p[:, 64:128])

        # ---- attention: S^T = K Q^T per j-block, exp, O^T_ext = V_ext^T P^T ----
        ot_raw = big.tile([65, 2, N], F32R, tag="slotB")   # row 64 = softmax sums
        groups = []
        jb0 = 0
        while jb0 < NB:
            g = min(SG, NB - jb0)
            groups.append((jb0, g))
            jb0 += g
        with tc.tile_pool(name="ps_att", bufs=1, space="PSUM") as ps_att, \
             tc.tile_pool(name="pt_sb", bufs=3) as ptp:
            for ic in range(NIC):
                po0 = ps_att.tile([65, 512], F32, tag="po0")
                po1 = ps_att.tile([65, 512], F32, tag="po1")
                po = [po0, po1]
                for jb0, g in groups:
                    for hl in range(2):
                        ps = ps_att.tile([128, SG, 512], F32, tag=f"ps{hl}")
                        for k in range(g):
                            jb = jb0 + k
                            nc.tensor.matmul(
                                ps[:, k, :],
                                kt[64 * hl:64 * hl + 64, 128 * jb:128 * jb + 128],
                                qt[64 * hl:64 * hl + 64, IC * ic:IC * ic + IC],
                                start=True, stop=True)
                        pt = ptp.tile([128, SG, 512], F32R, tag="pt")
                        nc.scalar.activation(out=pt[:, 0:g, :], in_=ps[:, 0:g, :],
                                             func=ACTF.Exp, scale=SCALE)
                        for k in range(g):
                            jb = jb0 + k
                            nc.tensor.matmul(
                                po[hl], v_ext[:, jb, 65 * hl:65 * hl + 65], pt[:, k, :],
                                start=(jb == 0), stop=(jb == NB - 1))
                for hl in range(2):
                    v.tensor_copy(out=ot_raw[:, hl, IC * ic:IC * ic + IC],
                                  in_=po[hl][0:65, :])

        # ---- softmax denominators -> natural layout [128, NB] per head ----
        with tc.tile_pool(name="fin", bufs=1) as fin, \
             tc.tile_pool(name="otmp", bufs=3) as otp:
            rr = []
            with tc.tile_pool(name="ps_r", bufs=1, space="PSUM") as ps_r, \
                 tc.tile_pool(name="drs", bufs=1, space="DRAM") as drs:
                for hl in range(2):
                    scr = drs.tile([1, N], F32, tag=f"scr{hl}")
                    nc.sync.dma_start(out=scr, in_=ot_raw.bitcast(F32)[64:65, hl, :])
                    rs = fin.tile([NB, 128], F32, tag=f"rs{hl}")
                    nc.sync.dma_start(
                        out=rs, in_=scr.rearrange("p (a b) -> (p a) b", b=128))
                    rc = fin.tile([NB, 128], F32, tag=f"rc{hl}")
                    v.reciprocal(out=rc, in_=rs)
                    pr = ps_r.tile([128, NB], F32, tag=f"pr{hl}")
                    nc.tensor.transpose(pr, rc, ident[0:NB, 0:NB])
                    t = fin.tile([128, NB], F32, tag=f"rr{hl}")
                    v.tensor_copy(out=t, in_=pr)
                    rr.append(t)

            # ---- per-head out-proj, scale by 1/sums, + flag*(x*A+B) residual ----
            out_sb = big.tile([128, NB, C], F32, tag="slotD")
            ps_o = stk.enter_context(tc.tile_pool(name="ps_out", bufs=3, space="PSUM"))
            for blk in range(NB):
                p0 = ps_o.tile([128, 256], F32, tag="p0")
                nc.tensor.matmul(p0, ot_raw[0:64, 0, 128 * blk:128 * blk + 128],
                                 wout_r[0], start=True, stop=True)
                p1 = ps_o.tile([128, 256], F32, tag="p1")
                nc.tensor.matmul(p1, ot_raw[0:64, 1, 128 * blk:128 * blk + 128],
                                 wout_r[1], start=True, stop=True)
                tmp = otp.tile([128, 256], F32, tag="tmp")
                v.scalar_tensor_tensor(out=tmp, in0=p0, scalar=rr[0][:, blk:blk + 1],
                                       in1=abf[:, 256:512], op0=ALU.mult, op1=ALU.add)
                v.scalar_tensor_tensor(out=tmp, in0=p1, scalar=rr[1][:, blk:blk + 1],
                                       in1=tmp, op0=ALU.mult, op1=ALU.add)
                t2 = otp.tile([128, 256], F32, tag="t2")
                v.tensor_mul(out=t2, in0=x_nat[:, blk, :], in1=abf[:, 0:256])
                v.tensor_add(out=out_sb[:, blk, :], in0=t2, in1=tmp)
            for k in range(4):
                nc.sync.dma_start(out=out_ap[:, 8 * k:8 * k + 8, :],
                                  in_=out_sb[:, 8 * k:8 * k + 8, :])


def _shards(x, gn_scale, gn_bias, w_qkv, w_out):
    maps = []
    for c in range(8):
        b, p = c // 2, c % 2
        h0 = 2 * p
        # reference: qkv.reshape(B, N, NH, 3*HD) then split -> head h uses
        # columns [192h:192h+64]=q, [192h+64:192h+128]=k, [192h+128:192h+192]=v
        cols = []
        for part in range(3):          # q, k, v
            for hh in (h0, h0 + 1):
                base = 3 * HD * hh + HD * part
                cols.append(w_qkv[:, base:base + HD])
        wq = np.concatenate(cols, axis=1)
        maps.append({
            "x": np.ascontiguousarray(x[b].reshape(N, C), dtype=np.float32),
            "wqkv": np.ascontiguousarray(wq, dtype=np.float32),
            "wout": np.ascontiguousarray(w_out[128 * p:128 * p + 128, :], dtype=np.float32),
            "gns": np.ascontiguousarray(gn_scale, dtype=np.float32),
            "gnb": np.ascontiguousarray(gn_bias, dtype=np.float32),
            "flag": np.full((128, 1), 1.0 if p == 0 else 0.0, dtype=np.float32),
        })
    return maps


def kernel(x, gn_scale, gn_bias, w_qkv, w_out, b_out):
    x = np.asarray(x, dtype=np.float32)
    gn_scale = np.asarray(gn_scale, dtype=np.float32)
    gn_bias = np.asarray(gn_bias, dtype=np.float32)
    w_qkv = np.asarray(w_qkv, dtype=np.float32)
    w_out = np.asarray(w_out, dtype=np.float32)
    b_out = np.asarray(b_out, dtype=np.float32)

    if "nc" not in _CACHED:
        _CACHED["nc"] = _build()
    nc = _CACHED["nc"]
    in_maps = _shards(x, gn_scale, gn_bias, w_qkv, w_out)
    res = bass_utils.run_bass_kernel_spmd(nc, in_maps, core_ids=list(range(8)))
    out = np.empty((B, H, W, C), dtype=np.float32)
    for b in range(B):
        s = res.results[2 * b]["out"] + res.results[2 * b + 1]["out"] + b_out[None, :]
        out[b] = s.reshape(H, W, C)
    return out


if __name__ == "__main__":
    rng = np.random.default_rng(0)
    x = rng.standard_normal((B, H, W, C)).astype(np.float32)
    out = kernel(x, np.ones(C, np.float32), np.zeros(C, np.float32),
                 (rng.standard_normal((C, 3 * C)) * C ** -0.5).astype(np.float32),
                 (rng.standard_normal((C, C)) * C ** -0.5).astype(np.float32),
                 np.zeros(C, np.float32))
    print(out.shape, out.dtype)


# revision 37
# speedup vs baseline: 6274599.4760x; 6274599.4760x over previous
"""AttentionBlock (GroupNorm + 4-head self-attention + out-proj + residual)
as a Bass/Tile kernel for 8 Trainium2 NeuronCores.

Sharding: 8 cores = 4 batches x 2 head-pairs. Core c handles batch c//2 and
heads {2*(c%2), 2*(c%2)+1}. Each core computes GroupNorm + QKV for its batch
(duplicated across the 2 cores of a batch), attention for its 2 heads, and a
partial output projection. Host sums the two partials per batch and adds b_out.

Layout strategy: matmuls contract over partitions, so xn lives in transposed
[C, N] layout (f32r) for QKV; attention runs in S^T orientation ([j, i]) so
softmax row sums come from a ones-column appended to V in the PV matmul; the
output projection contracts over head columns taking O^T directly, landing in
natural [N, C] layout where the softmax division is a per-partition scalar.
SBUF slots are aliased via shared pool tags across kernel phases.
"""

import sys

sys.path.insert(0, "/opt/trn_rl_repo")

from contextlib import ExitStack

import numpy as np

import concourse.bass as bass
import concourse.bacc as bacc
import concourse.tile as tile
from concourse import mybir
from concourse import bass_utils
from concourse.masks import make_identity

F32 = mybir.dt.float32
F32R = mybir.dt.float32r
ALU = mybir.AluOpType
ACTF = mybir.ActivationFunctionType

B, H, W, C = 4, 64, 64, 256
N = H * W               # 4096
NB = N // 128           # 32 natural row-blocks
NH = 4                  # heads in reference
HD = C // NH            # 64 head dim
NG = 8                  # groupnorm groups
GS = C // NG            # 32 channels per group
EPS = 1e-5
SCALE = C ** -0.5       # 1/16 attention scale
IC = 512                # attention i-chunk
NIC = N // IC           # 8
SG = 3                  # S^T psum banks per exp call

_CACHED = {}


def _build(repeat=1):
    nc = bacc.Bacc("TRN2", target_bir_lowering=False, debug=False, num_devices=8)

    x_d = nc.dram_tensor("x", [N, C], F32, kind="ExternalInput")
    wqkv_d = nc.dram_tensor("wqkv", [C, 384], F32, kind="ExternalInput")
    wout_d = nc.dram_tensor("wout", [128, C], F32, kind="ExternalInput")
    gns_d = nc.dram_tensor("gns", [C], F32, kind="ExternalInput")
    gnb_d = nc.dram_tensor("gnb", [C], F32, kind="ExternalInput")
    flag_d = nc.dram_tensor("flag", [128, 1], F32, kind="ExternalInput")
    out_d = nc.dram_tensor("out", [N, C], F32, kind="ExternalOutput")

    with tile.TileContext(nc) as tc:
        for _ in range(repeat):
            _body(tc, nc, x_d, wqkv_d, wout_d, gns_d, gnb_d, flag_d, out_d)
    nc.compile()
    return nc


def _body(tc, nc, x_d, wqkv_d, wout_d, gns_d, gnb_d, flag_d, out_d):
    v = nc.vector
    x_ap = x_d.ap().rearrange("(a b) c -> b a c", b=128)      # [128, 32, 256]
    out_ap = out_d.ap().rearrange("(a b) c -> b a c", b=128)

    with ExitStack() as stk:
        const = stk.enter_context(tc.tile_pool(name="const", bufs=1))
        big = stk.enter_context(tc.tile_pool(name="big", bufs=1))

        # ---- constants ----
        ident = const.tile([128, 128], F32)
        make_identity(nc, ident)
        ones1_128 = const.tile([1, 128], F32)
        nc.gpsimd.memset(ones1_128, 1.0)
        ones_f = const.tile([128, NB], F32)
        nc.gpsimd.memset(ones_f, 1.0)
        ind8 = []                 # per ct: [128, 8], partition q -> col 1 at (128ct+q)//32
        for ct in range(2):
            t = const.tile([128, 8], F32, tag=f"ind8_{ct}")
            nc.gpsimd.memset(t, 0.0)
            for g in range(4):
                nc.gpsimd.memset(t[32 * g:32 * g + 32, 4 * ct + g:4 * ct + g + 1], 1.0)
            ind8.append(t)
        indt = []                 # per ct: [8, 128] group -> channels of ct
        for ct in range(2):
            t = const.tile([8, 128], F32, tag=f"indt{ct}")
            nc.gpsimd.memset(t, 0.0)
            nc.gpsimd.affine_select(
                out=t.rearrange("p (a b) -> p a b", a=4), in_=t.rearrange("p (a b) -> p a b", a=4),
                compare_op=ALU.not_equal, fill=1.0, base=-4 * ct,
                pattern=[[-1, 4], [0, 32]], channel_multiplier=1)
            indt.append(t)
        ind_row = const.tile([8, 256], F32)   # group g -> channel row mask
        nc.gpsimd.memset(ind_row, 0.0)
        nc.gpsimd.affine_select(
            out=ind_row.rearrange("p (a b) -> p a b", a=8),
            in_=ind_row.rearrange("p (a b) -> p a b", a=8),
            compare_op=ALU.not_equal, fill=1.0, base=0,
            pattern=[[-1, 8], [0, 32]], channel_multiplier=1)

        gns_col = const.tile([128, 2], F32)
        nc.sync.dma_start(out=gns_col, in_=gns_d.ap().rearrange("(a b) -> b a", b=128))
        gnb_col = const.tile([128, 2], F32)
        nc.sync.dma_start(out=gnb_col, in_=gnb_d.ap().rearrange("(a b) -> b a", b=128))
        gns_row = const.tile([1, 256], F32)
        nc.sync.dma_start(out=gns_row, in_=gns_d.ap()[None, :])
        gnb_row = const.tile([1, 256], F32)
        nc.sync.dma_start(out=gnb_row, in_=gnb_d.ap()[None, :])
        flag_col = const.tile([128, 1], F32)
        nc.sync.dma_start(out=flag_col, in_=flag_d.ap())
        abf = const.tile([128, 512], F32)     # flag * (A | B) broadcast, set later
        wout_r = []
        for hl in range(2):
            wf = const.tile([64, 256], F32, tag=f"wf{hl}")
            nc.sync.dma_start(out=wf, in_=wout_d.ap()[64 * hl:64 * hl + 64, :])
            wr = const.tile([64, 256], F32R, tag=f"wr{hl}")
            v.tensor_copy(out=wr, in_=wf)
            wout_r.append(wr)
        wq_f = const.tile([128, 2, 384], F32)
        nc.sync.dma_start(out=wq_f, in_=wqkv_d.ap().rearrange("(a p) n -> p a n", p=128))
        wq_r = const.tile([128, 2, 384], F32R)
        v.tensor_copy(out=wq_r, in_=wq_f)

        # ---- persistent slots (aliased across phases via tags) ----
        x_nat = big.tile([128, NB, C], F32, tag="slotA")          # whole kernel
        for k in range(4):
            nc.sync.dma_start(out=x_nat[:, 8 * k:8 * k + 8, :],
                              in_=x_ap[:, 8 * k:8 * k + 8, :])

        # ---- transpose x -> xT [c, i] ----
        xt = big.tile([128, 2, N], F32, tag="slotB")              # -> ot_raw later
        with tc.tile_pool(name="ps_t", bufs=6, space="PSUM") as ps_t:
            for blk in range(NB):
                for ct in range(2):
                    p = ps_t.tile([128, 128], F32, tag="pt")
                    nc.tensor.transpose(p, x_nat[:, blk, 128 * ct:128 * ct + 128], ident)
                    if blk % 2 == 0:
                        v.tensor_copy(out=xt[:, ct, 128 * blk:128 * blk + 128], in_=p)
                    else:
                        nc.scalar.copy(out=xt[:, ct, 128 * blk:128 * blk + 128], in_=p)

        # ---- groupnorm stats + normalize ----
        xnt = big.tile([128, 2, N], F32R, tag="slotD")            # -> out_sb later
        with tc.tile_pool(name="stats", bufs=1) as stp, \
             tc.tile_pool(name="ps_s", bufs=1, space="PSUM") as ps_s:
            abrow = stp.tile([1, 512], F32)
            a_col = stp.tile([128, 2], F32)
            b_col = stp.tile([128, 2], F32)
            gp8 = ps_s.tile([8, 2], F32)
            for ct in range(2):
                bst = stp.tile([128, 8, 6], F32, tag="bst")
                for s in range(8):
                    v.bn_stats(out=bst[:, s, :], in_=xt[:, ct, 512 * s:512 * s + 512])
                mv = stp.tile([128, 2], F32, tag="mv")
                v.bn_aggr(out=mv, in_=bst)
                mm = stp.tile([128, 1], F32, tag="mm")
                v.tensor_scalar(out=mm, in0=mv[:, 0:1], scalar1=mv[:, 0:1], scalar2=None,
                                op0=ALU.mult)
                m1m2 = stp.tile([128, 2], F32, tag="m1m2")
                v.tensor_copy(out=m1m2[:, 0:1], in_=mv[:, 0:1])
                v.tensor_scalar(out=m1m2[:, 1:2], in0=mv[:, 1:2], scalar1=mm, scalar2=None,
                                op0=ALU.add)
                nc.tensor.matmul(gp8, ind8[ct], m1m2, start=(ct == 0), stop=(ct == 1))
            # group stats (gp8 cols: [sum mean_c, sum m2_c]) -> mean_g, rstd_g [8,1]
            mg8 = stp.tile([8, 1], F32)
            v.tensor_scalar(out=mg8, in0=gp8[:, 0:1], scalar1=1.0 / GS, scalar2=None,
                            op0=ALU.mult)
            e2g = stp.tile([8, 1], F32)
            v.tensor_scalar(out=e2g, in0=gp8[:, 1:2], scalar1=1.0 / GS, scalar2=None,
                            op0=ALU.mult)
            varg = stp.tile([8, 1], F32)
            v.tensor_scalar(out=varg, in0=mg8, scalar1=mg8, scalar2=None, op0=ALU.mult)
            v.tensor_sub(out=varg, in0=e2g, in1=varg)
            epst = stp.tile([8, 1], F32)
            v.memset(epst, EPS)
            stdg = stp.tile([8, 1], F32)
            nc.scalar.activation(out=stdg, in_=varg, func=ACTF.Sqrt, bias=epst)
            rstdg = stp.tile([8, 1], F32)
            v.reciprocal(out=rstdg, in_=stdg)
            grp2 = stp.tile([8, 2], F32)
            v.tensor_copy(out=grp2[:, 0:1], in_=mg8)
            v.tensor_copy(out=grp2[:, 1:2], in_=rstdg)
            # per-channel A,B in column layout (for xnt)
            for ct in range(2):
                bc = ps_s.tile([128, 2], F32, tag="bc")
                nc.tensor.matmul(bc, indt[ct], grp2, start=True, stop=True)
                v.tensor_mul(out=a_col[:, ct:ct + 1], in0=bc[:, 1:2],
                             in1=gns_col[:, ct:ct + 1])
                mA = stp.tile([128, 1], F32, tag="mA")
                v.tensor_mul(out=mA, in0=bc[:, 0:1], in1=a_col[:, ct:ct + 1])
                v.tensor_sub(out=b_col[:, ct:ct + 1], in0=gnb_col[:, ct:ct + 1], in1=mA)
            # per-channel A,B row layout -> flag * broadcast [128, A|B]
            mr_m = ps_s.tile([1, 256], F32, tag="mr_m")
            nc.tensor.matmul(mr_m, mg8, ind_row, start=True, stop=True)
            mr_r = ps_s.tile([1, 256], F32, tag="mr_r")
            nc.tensor.matmul(mr_r, rstdg, ind_row, start=True, stop=True)
            v.tensor_mul(out=abrow[:, 0:256], in0=mr_r, in1=gns_row)
            marow = stp.tile([1, 256], F32)
            v.tensor_mul(out=marow, in0=mr_m, in1=abrow[:, 0:256])
            v.tensor_sub(out=abrow[:, 256:512], in0=gnb_row, in1=marow)
            abp = ps_s.tile([128, 512], F32, tag="abp")
            nc.tensor.matmul(abp, ones1_128, abrow, start=True, stop=True)
            v.tensor_scalar_mul(out=abf, in0=abp, scalar1=flag_col)

            # normalize transposed copy in place: xnt = xt * A + B (f32r)
            for ct in range(2):
                v.tensor_scalar(out=xnt[:, ct, :], in0=xt[:, ct, :],
                                scalar1=a_col[:, ct:ct + 1], scalar2=b_col[:, ct:ct + 1],
                                op0=ALU.mult, op1=ALU.add)

        # ---- QKV^T = wqkv_sh^T @ xn^T : q/k/v each [128, N] ----
        qt = big.tile([128, N], F32R, tag="slotQ")
        kt = big.tile([128, N], F32R, tag="slotK")
        vt = big.tile([128, N], F32, tag="slotC")
        with tc.tile_pool(name="ps_qkv", bufs=4, space="PSUM") as ps_q:
            for qi, dst in ((0, qt), (1, kt), (2, vt)):
                for ic in range(NIC):
                    p = ps_q.tile([128, 512], F32, tag="pq")
                    for ct in range(2):
                        nc.tensor.matmul(p, wq_r[:, ct, 128 * qi:128 * qi + 128],
                                         xnt[:, ct, IC * ic:IC * ic + IC],
                                         start=(ct == 0), stop=(ct == 1))
                    if qi == 2:   # vt is plain f32: ACT can copy it (idle here)
                        nc.scalar.copy(out=dst[:, IC * ic:IC * ic + IC], in_=p)
                    else:         # f32r rounding copies stay on DVE
                        v.tensor_copy(out=dst[:, IC * ic:IC * ic + IC], in_=p)

        # ---- V^T -> V natural with ones columns ----
        v_ext = big.tile([128, NB, 130], F32R, tag="slotV")
        v.tensor_copy(out=v_ext[:, :, 64:65], in_=ones_f[:, :, None])
        v.tensor_copy(out=v_ext[:, :, 129:130], in_=ones_f[:, :, None])
        with tc.tile_pool(name="ps_vt", bufs=4, space="PSUM") as ps_v:
            for jb in range(NB):
                p = ps_v.tile([128, 128], F32, tag="pv")
                nc.tensor.transpose(p, vt[:, 128 * jb:128 * jb + 128], ident)
                v.tensor_copy(out=v_ext[:, jb, 0:64], in_=p[:, 0:64])
                v.tensor_copy(out=v_ext[:, jb, 65:129], in_=p[:, 64:128])

        # ---- attention: S^T = K Q^T per j-block, exp, O^T_ext = V_ext^T P^T;
        #      the tail (sums, out-proj, residual, store) pipelines per i-chunk ----
        ot_raw = big.tile([65, 2, N], F32R, tag="slotB")   # row 64 = softmax sums
        out_sb = big.tile([128, NB, C], F32, tag="slotD")
        groups = []
        jb0 = 0
        while jb0 < NB:
            g = min(SG, NB - jb0)
            groups.append((jb0, g))
            jb0 += g
        with tc.tile_pool(name="ps_att", bufs=1, space="PSUM") as ps_att, \
             tc.tile_pool(name="pt_sb", bufs=3) as ptp, \
             tc.tile_pool(name="fin", bufs=1) as fin, \
             tc.tile_pool(name="otmp", bufs=3) as otp, \
             tc.tile_pool(name="drs", bufs=1, space="DRAM") as drs:
            scr0 = drs.tile([1, N], F32, tag="scr0")
            scr1 = drs.tile([1, N], F32, tag="scr1")
            scr = [scr0, scr1]
            rraw0 = fin.tile([128, NB], F32, tag="rraw0")
            rraw1 = fin.tile([128, NB], F32, tag="rraw1")
            rr0 = fin.tile([128, NB], F32, tag="rr0")
            rr1 = fin.tile([128, NB], F32, tag="rr1")
            rraw = [rraw0, rraw1]
            rr = [rr0, rr1]
            for ic in range(NIC):
                po0 = ps_att.tile([65, 512], F32, tag="po0")
                po1 = ps_att.tile([65, 512], F32, tag="po1")
                po = [po0, po1]
                for jb0, g in groups:
                    for hl in range(2):
                        ps = ps_att.tile([128, SG, 512], F32, tag=f"ps{hl}")
                        for k in range(g):
                            jb = jb0 + k
                            nc.tensor.matmul(
                                ps[:, k, :],
                                kt[64 * hl:64 * hl + 64, 128 * jb:128 * jb + 128],
                                qt[64 * hl:64 * hl + 64, IC * ic:IC * ic + IC],
                                start=True, stop=True)
                        pt = ptp.tile([128, SG, 512], F32R, tag="pt")
                        nc.scalar.activation(out=pt[:, 0:g, :], in_=ps[:, 0:g, :],
                                             func=ACTF.Exp, scale=SCALE)
                        for k in range(g):
                            jb = jb0 + k
                            nc.tensor.matmul(
                                po[hl], v_ext[:, jb, 65 * hl:65 * hl + 65], pt[:, k, :],
                                start=(jb == 0), stop=(jb == NB - 1))
                # epilogue for this i-chunk: stash O^T_ext, sums -> 1/sums natural
                for hl in range(2):
                    v.tensor_copy(out=ot_raw[:, hl, IC * ic:IC * ic + IC],
                                  in_=po[hl][0:65, :])
                    nc.sync.dma_start(out=scr[hl][:, IC * ic:IC * ic + IC],
                                      in_=ot_raw.bitcast(F32)[64:65, hl,
                                                              IC * ic:IC * ic + IC])
                    nc.sync.dma_start(
                        out=rraw[hl][:, 4 * ic:4 * ic + 4],
                        in_=scr[hl][:, IC * ic:IC * ic + IC].rearrange(
                            "p (a b) -> (p b) a", a=4))
                    v.reciprocal(out=rr[hl][:, 4 * ic:4 * ic + 4],
                                 in_=rraw[hl][:, 4 * ic:4 * ic + 4])
                # out-proj + softmax scale + flag*(x*A+B) residual for these rows
                for blk in range(4 * ic, 4 * ic + 4):
                    p0 = ps_att.tile([128, 256], F32, tag="po0")
                    nc.tensor.matmul(p0, ot_raw[0:64, 0, 128 * blk:128 * blk + 128],
                                     wout_r[0], start=True, stop=True)
                    p1 = ps_att.tile([128, 256], F32, tag="po1")
                    nc.tensor.matmul(p1, ot_raw[0:64, 1, 128 * blk:128 * blk + 128],
                                     wout_r[1], start=True, stop=True)
                    tmp = otp.tile([128, 256], F32, tag="tmp")
                    v.scalar_tensor_tensor(out=tmp, in0=p0,
                                           scalar=rr[0][:, blk:blk + 1],
                                           in1=abf[:, 256:512],
                                           op0=ALU.mult, op1=ALU.add)
                    v.scalar_tensor_tensor(out=tmp, in0=p1,
                                           scalar=rr[1][:, blk:blk + 1],
                                           in1=tmp, op0=ALU.mult, op1=ALU.add)
                    t2 = otp.tile([128, 256], F32, tag="t2")
                    v.tensor_mul(out=t2, in0=x_nat[:, blk, :], in1=abf[:, 0:256])
                    v.tensor_add(out=out_sb[:, blk, :], in0=t2, in1=tmp)
                nc.sync.dma_start(out=out_ap[:, 4 * ic:4 * ic + 4, :],
                                  in_=out_sb[:, 4 * ic:4 * ic + 4, :])


def _shards(x, gn_scale, gn_bias, w_qkv, w_out):
    maps = []
    for c in range(8):
        b, p = c // 2, c % 2
        h0 = 2 * p
        # reference: qkv.reshape(B, N, NH, 3*HD) then split -> head h uses
        # columns [192h:192h+64]=q, [192h+64:192h+128]=k, [192h+128:192h+192]=v
        cols = []
        for part in range(3):          # q, k, v
            for hh in (h0, h0 + 1):
                base = 3 * HD * hh + HD * part
                cols.append(w_qkv[:, base:base + HD])
        wq = np.concatenate(cols, axis=1)
        maps.append({
            "x": np.ascontiguousarray(x[b].reshape(N, C), dtype=np.float32),
            "wqkv": np.ascontiguousarray(wq, dtype=np.float32),
            "wout": np.ascontiguousarray(w_out[128 * p:128 * p + 128, :], dtype=np.float32),
            "gns": np.ascontiguousarray(gn_scale, dtype=np.float32),
            "gnb": np.ascontiguousarray(gn_bias, dtype=np.float32),
            "flag": np.full((128, 1), 1.0 if p == 0 else 0.0, dtype=np.float32),
        })
    return maps


def _make_runner(repeat=1):
    """Build the program once and return fn(in_maps) -> list of out arrays."""
    import jax
    from jax.sharding import Mesh, PartitionSpec
    from jax.experimental.shard_map import shard_map
    from concourse import bass2jax
    import concourse.mybir as mb

    nc = _build(repeat=repeat)
    bass2jax.install_neuronx_cc_hook()
    partition_name = nc.partition_id_tensor.name if nc.partition_id_tensor else None
    in_names, out_names, out_avals, zero_shapes = [], [], [], []
    for alloc in nc.m.functions[0].allocations:
        if not isinstance(alloc, mb.MemoryLocationSet):
            continue
        name = alloc.memorylocations[0].name
        if alloc.kind == "ExternalInput":
            if name != partition_name:
                in_names.append(name)
        elif alloc.kind == "ExternalOutput":
            shape = tuple(alloc.tensor_shape)
            dtype = mb.dt.np(alloc.dtype)
            out_names.append(name)
            out_avals.append(jax.core.ShapedArray(shape, dtype))
            zero_shapes.append((shape, dtype))
    n_params = len(in_names)
    all_names = list(in_names) + list(out_names)
    if partition_name is not None:
        all_names.append(partition_name)

    def _bass_body(*args):
        operands = list(args)
        if partition_name is not None:
            operands.append(bass2jax.partition_id_tensor())
        outs = bass2jax._bass_exec_p.bind(
            *operands,
            out_avals=tuple(out_avals),
            in_names=tuple(all_names),
            out_names=tuple(out_names),
            lowering_input_output_aliases=(),
            sim_require_finite=True,
            sim_require_nnan=True,
            nc=nc,
        )
        return tuple(outs)

    n_outs = len(out_avals)
    donate = tuple(range(n_params, n_params + n_outs))
    devices = jax.devices()[:8]
    mesh = Mesh(np.asarray(devices), ("core",))
    sharded = jax.jit(
        shard_map(_bass_body, mesh=mesh,
                  in_specs=(PartitionSpec("core"),) * (n_params + n_outs),
                  out_specs=(PartitionSpec("core"),) * n_outs,
                  check_rep=False),
        donate_argnums=donate, keep_unused=True)

    def run(in_maps):
        concat_in = [np.concatenate([np.asarray(m[nm]) for m in in_maps], axis=0)
                     for nm in in_names]
        concat_zero = [np.zeros((8 * s[0], *s[1:]), d) for s, d in zero_shapes]
        out_arrs = sharded(*concat_in, *concat_zero)
        outs = {}
        for i, nm in enumerate(out_names):
            outs[nm] = np.asarray(out_arrs[i]).reshape(8, *zero_shapes[i][0])
        return outs

    return run


def kernel(x, gn_scale, gn_bias, w_qkv, w_out, b_out):
    x = np.asarray(x, dtype=np.float32)
    gn_scale = np.asarray(gn_scale, dtype=np.float32)
    gn_bias = np.asarray(gn_bias, dtype=np.float32)
    w_qkv = np.asarray(w_qkv, dtype=np.float32)
    w_out = np.asarray(w_out, dtype=np.float32)
    b_out = np.asarray(b_out, dtype=np.float32)

    if "run" not in _CACHED:
        _CACHED["run"] = _make_runner()
    in_maps = _shards(x, gn_scale, gn_bias, w_qkv, w_out)
    outs = _CACHED["run"](in_maps)["out"]
    out = np.empty((B, H, W, C), dtype=np.float32)
    for b in range(B):
        s = outs[2 * b] + outs[2 * b + 1] + b_out[None, :]
        out[b] = s.reshape(H, W, C)
    return out


if __name__ == "__main__":
    rng = np.random.default_rng(0)
    x = rng.standard_normal((B, H, W, C)).astype(np.float32)
    out = kernel(x, np.ones(C, np.float32), np.zeros(C, np.float32),
                 (rng.standard_normal((C, 3 * C)) * C ** -0.5).astype(np.float32),
                 (rng.standard_normal((C, C)) * C ** -0.5).astype(np.float32),
                 np.zeros(C, np.float32))
    print(out.shape, out.dtype)


# revision 38
# speedup vs baseline: 6627008.4877x; 1.0562x over previous
"""AttentionBlock (GroupNorm + 4-head self-attention + out-proj + residual)
as a Bass/Tile kernel for 8 Trainium2 NeuronCores.

Sharding: 8 cores = 4 batches x 2 head-pairs. Core c handles batch c//2 and
heads {2*(c%2), 2*(c%2)+1}. Each core computes GroupNorm + QKV for its batch
(duplicated across the 2 cores of a batch), attention for its 2 heads, and a
partial output projection. Host sums the two partials per batch and adds b_out.

Layout strategy: matmuls contract over partitions, so xn lives in transposed
[C, N] layout (f32r) for QKV; attention runs in S^T orientation ([j, i]) so
softmax row sums come from a ones-column appended to V in the PV matmul; the
output projection contracts over head columns taking O^T directly, landing in
natural [N, C] layout where the softmax division is a per-partition scalar.
SBUF slots are aliased via shared pool tags across kernel phases.
"""

import sys

sys.path.insert(0, "/opt/trn_rl_repo")

from contextlib import ExitStack

import numpy as np

import concourse.bass as bass
import concourse.bacc as bacc
import concourse.tile as tile
from concourse import mybir
from concourse import bass_utils
from concourse.masks import make_identity

F32 = mybir.dt.float32
F32R = mybir.dt.float32r
ALU = mybir.AluOpType
ACTF = mybir.ActivationFunctionType

B, H, W, C = 4, 64, 64, 256
N = H * W               # 4096
NB = N // 128           # 32 natural row-blocks
NH = 4                  # heads in reference
HD = C // NH            # 64 head dim
NG = 8                  # groupnorm groups
GS = C // NG            # 32 channels per group
EPS = 1e-5
SCALE = C ** -0.5       # 1/16 attention scale
IC = 512                # attention i-chunk
NIC = N // IC           # 8
SG = 3                  # S^T psum banks per exp call

_CACHED = {}


def _build(repeat=1):
    nc = bacc.Bacc("TRN2", target_bir_lowering=False, debug=False, num_devices=8)

    x_d = nc.dram_tensor("x", [N, C], F32, kind="ExternalInput")
    wqkv_d = nc.dram_tensor("wqkv", [C, 384], F32, kind="ExternalInput")
    wout_d = nc.dram_tensor("wout", [128, C], F32, kind="ExternalInput")
    gns_d = nc.dram_tensor("gns", [C], F32, kind="ExternalInput")
    gnb_d = nc.dram_tensor("gnb", [C], F32, kind="ExternalInput")
    flag_d = nc.dram_tensor("flag", [128, 1], F32, kind="ExternalInput")
    out_d = nc.dram_tensor("out", [N, C], F32, kind="ExternalOutput")

    with tile.TileContext(nc) as tc:
        for _ in range(repeat):
            _body(tc, nc, x_d, wqkv_d, wout_d, gns_d, gnb_d, flag_d, out_d)
    nc.compile()
    return nc


def _body(tc, nc, x_d, wqkv_d, wout_d, gns_d, gnb_d, flag_d, out_d):
    v = nc.vector
    x_ap = x_d.ap().rearrange("(a b) c -> b a c", b=128)      # [128, 32, 256]
    out_ap = out_d.ap().rearrange("(a b) c -> b a c", b=128)

    with ExitStack() as stk:
        const = stk.enter_context(tc.tile_pool(name="const", bufs=1))
        big = stk.enter_context(tc.tile_pool(name="big", bufs=1))

        # ---- constants ----
        ident = const.tile([128, 128], F32)
        make_identity(nc, ident)
        ones1_128 = const.tile([1, 128], F32)
        nc.gpsimd.memset(ones1_128, 1.0)
        ones_f = const.tile([128, NB], F32)
        nc.gpsimd.memset(ones_f, 1.0)
        ind8 = []                 # per ct: [128, 8], partition q -> col 1 at (128ct+q)//32
        for ct in range(2):
            t = const.tile([128, 8], F32, tag=f"ind8_{ct}")
            nc.gpsimd.memset(t, 0.0)
            for g in range(4):
                nc.gpsimd.memset(t[32 * g:32 * g + 32, 4 * ct + g:4 * ct + g + 1], 1.0)
            ind8.append(t)
        indt = []                 # per ct: [8, 128] group -> channels of ct
        for ct in range(2):
            t = const.tile([8, 128], F32, tag=f"indt{ct}")
            nc.gpsimd.memset(t, 0.0)
            nc.gpsimd.affine_select(
                out=t.rearrange("p (a b) -> p a b", a=4), in_=t.rearrange("p (a b) -> p a b", a=4),
                compare_op=ALU.not_equal, fill=1.0, base=-4 * ct,
                pattern=[[-1, 4], [0, 32]], channel_multiplier=1)
            indt.append(t)
        ind_row = const.tile([8, 256], F32)   # group g -> channel row mask
        nc.gpsimd.memset(ind_row, 0.0)
        nc.gpsimd.affine_select(
            out=ind_row.rearrange("p (a b) -> p a b", a=8),
            in_=ind_row.rearrange("p (a b) -> p a b", a=8),
            compare_op=ALU.not_equal, fill=1.0, base=0,
            pattern=[[-1, 8], [0, 32]], channel_multiplier=1)

        gns_col = const.tile([128, 2], F32)
        nc.sync.dma_start(out=gns_col, in_=gns_d.ap().rearrange("(a b) -> b a", b=128))
        gnb_col = const.tile([128, 2], F32)
        nc.sync.dma_start(out=gnb_col, in_=gnb_d.ap().rearrange("(a b) -> b a", b=128))
        gns_row = const.tile([1, 256], F32)
        nc.sync.dma_start(out=gns_row, in_=gns_d.ap()[None, :])
        gnb_row = const.tile([1, 256], F32)
        nc.sync.dma_start(out=gnb_row, in_=gnb_d.ap()[None, :])
        flag_col = const.tile([128, 1], F32)
        nc.sync.dma_start(out=flag_col, in_=flag_d.ap())
        abf = const.tile([128, 512], F32)     # flag * (A | B) broadcast, set later
        wout_r = []
        for hl in range(2):
            wf = const.tile([64, 256], F32, tag=f"wf{hl}")
            nc.sync.dma_start(out=wf, in_=wout_d.ap()[64 * hl:64 * hl + 64, :])
            wr = const.tile([64, 256], F32R, tag=f"wr{hl}")
            v.tensor_copy(out=wr, in_=wf)
            wout_r.append(wr)
        wq_f = const.tile([128, 2, 384], F32)
        nc.sync.dma_start(out=wq_f, in_=wqkv_d.ap().rearrange("(a p) n -> p a n", p=128))
        wq_r = const.tile([128, 2, 384], F32R)
        v.tensor_copy(out=wq_r, in_=wq_f)

        # ---- persistent slots (aliased across phases via tags) ----
        x_nat = big.tile([128, NB, C], F32, tag="slotA")          # whole kernel
        for k in range(4):
            nc.sync.dma_start(out=x_nat[:, 8 * k:8 * k + 8, :],
                              in_=x_ap[:, 8 * k:8 * k + 8, :])

        # ---- transpose x -> xT [c, i] ----
        xt = big.tile([128, 2, N], F32, tag="slotB")              # -> ot_raw later
        with tc.tile_pool(name="ps_t", bufs=6, space="PSUM") as ps_t:
            for blk in range(NB):
                for ct in range(2):
                    p = ps_t.tile([128, 128], F32, tag="pt")
                    nc.tensor.transpose(p, x_nat[:, blk, 128 * ct:128 * ct + 128], ident)
                    if blk % 2 == 0:
                        v.tensor_copy(out=xt[:, ct, 128 * blk:128 * blk + 128], in_=p)
                    else:
                        nc.scalar.copy(out=xt[:, ct, 128 * blk:128 * blk + 128], in_=p)

        # ---- groupnorm stats + normalize ----
        xnt = big.tile([128, 2, N], F32R, tag="slotD")            # -> out_sb later
        with tc.tile_pool(name="stats", bufs=1) as stp, \
             tc.tile_pool(name="ps_s", bufs=1, space="PSUM") as ps_s:
            abrow = stp.tile([1, 512], F32)
            a_col = stp.tile([128, 2], F32)
            b_col = stp.tile([128, 2], F32)
            gp8 = ps_s.tile([8, 2], F32)
            for ct in range(2):
                bst = stp.tile([128, 8, 6], F32, tag="bst")
                for s in range(8):
                    v.bn_stats(out=bst[:, s, :], in_=xt[:, ct, 512 * s:512 * s + 512])
                mv = stp.tile([128, 2], F32, tag="mv")
                v.bn_aggr(out=mv, in_=bst)
                mm = stp.tile([128, 1], F32, tag="mm")
                v.tensor_scalar(out=mm, in0=mv[:, 0:1], scalar1=mv[:, 0:1], scalar2=None,
                                op0=ALU.mult)
                m1m2 = stp.tile([128, 2], F32, tag="m1m2")
                v.tensor_copy(out=m1m2[:, 0:1], in_=mv[:, 0:1])
                v.tensor_scalar(out=m1m2[:, 1:2], in0=mv[:, 1:2], scalar1=mm, scalar2=None,
                                op0=ALU.add)
                nc.tensor.matmul(gp8, ind8[ct], m1m2, start=(ct == 0), stop=(ct == 1))
            # group stats (gp8 cols: [sum mean_c, sum m2_c]) -> mean_g, rstd_g [8,1]
            mg8 = stp.tile([8, 1], F32)
            v.tensor_scalar(out=mg8, in0=gp8[:, 0:1], scalar1=1.0 / GS, scalar2=None,
                            op0=ALU.mult)
            e2g = stp.tile([8, 1], F32)
            v.tensor_scalar(out=e2g, in0=gp8[:, 1:2], scalar1=1.0 / GS, scalar2=None,
                            op0=ALU.mult)
            varg = stp.tile([8, 1], F32)
            v.tensor_scalar(out=varg, in0=mg8, scalar1=mg8, scalar2=None, op0=ALU.mult)
            v.tensor_sub(out=varg, in0=e2g, in1=varg)
            epst = stp.tile([8, 1], F32)
            v.memset(epst, EPS)
            stdg = stp.tile([8, 1], F32)
            nc.scalar.activation(out=stdg, in_=varg, func=ACTF.Sqrt, bias=epst)
            rstdg = stp.tile([8, 1], F32)
            v.reciprocal(out=rstdg, in_=stdg)
            grp2 = stp.tile([8, 2], F32)
            v.tensor_copy(out=grp2[:, 0:1], in_=mg8)
            v.tensor_copy(out=grp2[:, 1:2], in_=rstdg)
            # per-channel A,B in column layout (for xnt)
            for ct in range(2):
                bc = ps_s.tile([128, 2], F32, tag="bc")
                nc.tensor.matmul(bc, indt[ct], grp2, start=True, stop=True)
                v.tensor_mul(out=a_col[:, ct:ct + 1], in0=bc[:, 1:2],
                             in1=gns_col[:, ct:ct + 1])
                mA = stp.tile([128, 1], F32, tag="mA")
                v.tensor_mul(out=mA, in0=bc[:, 0:1], in1=a_col[:, ct:ct + 1])
                v.tensor_sub(out=b_col[:, ct:ct + 1], in0=gnb_col[:, ct:ct + 1], in1=mA)
            # per-channel A,B row layout -> flag * broadcast [128, A|B]
            mr_m = ps_s.tile([1, 256], F32, tag="mr_m")
            nc.tensor.matmul(mr_m, mg8, ind_row, start=True, stop=True)
            mr_r = ps_s.tile([1, 256], F32, tag="mr_r")
            nc.tensor.matmul(mr_r, rstdg, ind_row, start=True, stop=True)
            v.tensor_mul(out=abrow[:, 0:256], in0=mr_r, in1=gns_row)
            marow = stp.tile([1, 256], F32)
            v.tensor_mul(out=marow, in0=mr_m, in1=abrow[:, 0:256])
            v.tensor_sub(out=abrow[:, 256:512], in0=gnb_row, in1=marow)
            abp = ps_s.tile([128, 512], F32, tag="abp")
            nc.tensor.matmul(abp, ones1_128, abrow, start=True, stop=True)
            v.tensor_scalar_mul(out=abf, in0=abp, scalar1=flag_col)

            # normalize transposed copy in place: xnt = xt * A + B (f32r)
            for ct in range(2):
                v.tensor_scalar(out=xnt[:, ct, :], in0=xt[:, ct, :],
                                scalar1=a_col[:, ct:ct + 1], scalar2=b_col[:, ct:ct + 1],
                                op0=ALU.mult, op1=ALU.add)

        # ---- QKV^T = wqkv_sh^T @ xn^T : q/k/v each [128, N] ----
        qt = big.tile([128, N], F32R, tag="slotQ")
        kt = big.tile([128, N], F32R, tag="slotK")
        vt = big.tile([128, N], F32, tag="slotC")
        with tc.tile_pool(name="ps_qkv", bufs=4, space="PSUM") as ps_q:
            for qi, dst in ((0, qt), (1, kt), (2, vt)):
                for ic in range(NIC):
                    p = ps_q.tile([128, 512], F32, tag="pq")
                    for ct in range(2):
                        nc.tensor.matmul(p, wq_r[:, ct, 128 * qi:128 * qi + 128],
                                         xnt[:, ct, IC * ic:IC * ic + IC],
                                         start=(ct == 0), stop=(ct == 1))
                    if qi == 2:   # vt is plain f32: ACT can copy it (idle here)
                        nc.scalar.copy(out=dst[:, IC * ic:IC * ic + IC], in_=p)
                    else:         # f32r rounding copies stay on DVE
                        v.tensor_copy(out=dst[:, IC * ic:IC * ic + IC], in_=p)

        # ---- V^T -> V natural with ones columns ----
        v_ext = big.tile([128, NB, 130], F32R, tag="slotV")
        v.tensor_copy(out=v_ext[:, :, 64:65], in_=ones_f[:, :, None])
        v.tensor_copy(out=v_ext[:, :, 129:130], in_=ones_f[:, :, None])
        with tc.tile_pool(name="ps_vt", bufs=4, space="PSUM") as ps_v:
            for jb in range(NB):
                p = ps_v.tile([128, 128], F32, tag="pv")
                nc.tensor.transpose(p, vt[:, 128 * jb:128 * jb + 128], ident)
                v.tensor_copy(out=v_ext[:, jb, 0:64], in_=p[:, 0:64])
                v.tensor_copy(out=v_ext[:, jb, 65:129], in_=p[:, 64:128])

        # ---- attention: S^T = K Q^T per j-block, exp, O^T_ext = V_ext^T P^T;
        #      the tail (sums, out-proj, residual, store) pipelines per i-chunk ----
        ot_raw = big.tile([65, 2, N], F32R, tag="slotB")   # row 64 = softmax sums
        out_sb = big.tile([128, NB, C], F32, tag="slotD")
        groups = []
        jb0 = 0
        while jb0 < NB:
            g = min(SG, NB - jb0)
            groups.append((jb0, g))
            jb0 += g
        with tc.tile_pool(name="ps_att", bufs=1, space="PSUM") as ps_att, \
             tc.tile_pool(name="pt_sb", bufs=3) as ptp, \
             tc.tile_pool(name="fin", bufs=1) as fin, \
             tc.tile_pool(name="otmp", bufs=3) as otp, \
             tc.tile_pool(name="drs", bufs=1, space="DRAM") as drs:
            scr0 = drs.tile([1, N], F32, tag="scr0")
            scr1 = drs.tile([1, N], F32, tag="scr1")
            scr = [scr0, scr1]
            rraw0 = fin.tile([128, NB], F32, tag="rraw0")
            rraw1 = fin.tile([128, NB], F32, tag="rraw1")
            rr0 = fin.tile([128, NB], F32, tag="rr0")
            rr1 = fin.tile([128, NB], F32, tag="rr1")
            rraw = [rraw0, rraw1]
            rr = [rr0, rr1]
            def tail(ic):
                # out-proj + softmax scale + flag*(x*A+B) residual for ic's rows
                for blk in range(4 * ic, 4 * ic + 4):
                    p0 = ps_att.tile([128, 256], F32, tag="po0")
                    nc.tensor.matmul(p0, ot_raw[0:64, 0, 128 * blk:128 * blk + 128],
                                     wout_r[0], start=True, stop=True)
                    p1 = ps_att.tile([128, 256], F32, tag="po1")
                    nc.tensor.matmul(p1, ot_raw[0:64, 1, 128 * blk:128 * blk + 128],
                                     wout_r[1], start=True, stop=True)
                    tmp = otp.tile([128, 256], F32, tag="tmp")
                    v.scalar_tensor_tensor(out=tmp, in0=p0,
                                           scalar=rr[0][:, blk:blk + 1],
                                           in1=abf[:, 256:512],
                                           op0=ALU.mult, op1=ALU.add)
                    v.scalar_tensor_tensor(out=tmp, in0=p1,
                                           scalar=rr[1][:, blk:blk + 1],
                                           in1=tmp, op0=ALU.mult, op1=ALU.add)
                    t2 = otp.tile([128, 256], F32, tag="t2")
                    v.tensor_mul(out=t2, in0=x_nat[:, blk, :], in1=abf[:, 0:256])
                    v.tensor_add(out=out_sb[:, blk, :], in0=t2, in1=tmp)
                nc.sync.dma_start(out=out_ap[:, 4 * ic:4 * ic + 4, :],
                                  in_=out_sb[:, 4 * ic:4 * ic + 4, :])

            for ic in range(NIC):
                po0 = ps_att.tile([65, 512], F32, tag="po0")
                po1 = ps_att.tile([65, 512], F32, tag="po1")
                po = [po0, po1]
                for jb0, g in groups:
                    for hl in range(2):
                        ps = ps_att.tile([128, SG, 512], F32, tag=f"ps{hl}")
                        for k in range(g):
                            jb = jb0 + k
                            nc.tensor.matmul(
                                ps[:, k, :],
                                kt[64 * hl:64 * hl + 64, 128 * jb:128 * jb + 128],
                                qt[64 * hl:64 * hl + 64, IC * ic:IC * ic + IC],
                                start=True, stop=True)
                        pt = ptp.tile([128, SG, 512], F32R, tag="pt")
                        nc.scalar.activation(out=pt[:, 0:g, :], in_=ps[:, 0:g, :],
                                             func=ACTF.Exp, scale=SCALE)
                        for k in range(g):
                            jb = jb0 + k
                            nc.tensor.matmul(
                                po[hl], v_ext[:, jb, 65 * hl:65 * hl + 65], pt[:, k, :],
                                start=(jb == 0), stop=(jb == NB - 1))
                # epilogue for this i-chunk: stash O^T_ext, sums -> 1/sums natural
                for hl in range(2):
                    v.tensor_copy(out=ot_raw[:, hl, IC * ic:IC * ic + IC],
                                  in_=po[hl][0:65, :])
                    nc.sync.dma_start(out=scr[hl][:, IC * ic:IC * ic + IC],
                                      in_=ot_raw.bitcast(F32)[64:65, hl,
                                                              IC * ic:IC * ic + IC])
                    nc.sync.dma_start(
                        out=rraw[hl][:, 4 * ic:4 * ic + 4],
                        in_=scr[hl][:, IC * ic:IC * ic + IC].rearrange(
                            "p (a b) -> (p b) a", a=4))
                    v.reciprocal(out=rr[hl][:, 4 * ic:4 * ic + 4],
                                 in_=rraw[hl][:, 4 * ic:4 * ic + 4])
                if ic > 0:
                    tail(ic - 1)     # one i-chunk behind: psum slots never block
            tail(NIC - 1)


def _shards(x, gn_scale, gn_bias, w_qkv, w_out):
    maps = []
    for c in range(8):
        b, p = c // 2, c % 2
        h0 = 2 * p
        # reference: qkv.reshape(B, N, NH, 3*HD) then split -> head h uses
        # columns [192h:192h+64]=q, [192h+64:192h+128]=k, [192h+128:192h+192]=v
        cols = []
        for part in range(3):          # q, k, v
            for hh in (h0, h0 + 1):
                base = 3 * HD * hh + HD * part
                cols.append(w_qkv[:, base:base + HD])
        wq = np.concatenate(cols, axis=1)
        maps.append({
            "x": np.ascontiguousarray(x[b].reshape(N, C), dtype=np.float32),
            "wqkv": np.ascontiguousarray(wq, dtype=np.float32),
            "wout": np.ascontiguousarray(w_out[128 * p:128 * p + 128, :], dtype=np.float32),
            "gns": np.ascontiguousarray(gn_scale, dtype=np.float32),
            "gnb": np.ascontiguousarray(gn_bias, dtype=np.float32),
            "flag": np.full((128, 1), 1.0 if p == 0 else 0.0, dtype=np.float32),
        })
    return maps


def _make_runner(repeat=1):
    """Build the program once and return fn(in_maps) -> list of out arrays."""
    import jax
    from jax.sharding import Mesh, PartitionSpec
    from jax.experimental.shard_map import shard_map
    from concourse import bass2jax
    import concourse.mybir as mb

    nc = _build(repeat=repeat)
    bass2jax.install_neuronx_cc_hook()
    partition_name = nc.partition_id_tensor.name if nc.partition_id_tensor else None
    in_names, out_names, out_avals, zero_shapes = [], [], [], []
    for alloc in nc.m.functions[0].allocations:
        if not isinstance(alloc, mb.MemoryLocationSet):
            continue
        name = alloc.memorylocations[0].name
        if alloc.kind == "ExternalInput":
            if name != partition_name:
                in_names.append(name)
        elif alloc.kind == "ExternalOutput":
            shape = tuple(alloc.tensor_shape)
            dtype = mb.dt.np(alloc.dtype)
            out_names.append(name)
            out_avals.append(jax.core.ShapedArray(shape, dtype))
            zero_shapes.append((shape, dtype))
    n_params = len(in_names)
    all_names = list(in_names) + list(out_names)
    if partition_name is not None:
        all_names.append(partition_name)

    def _bass_body(*args):
        operands = list(args)
        if partition_name is not None:
            operands.append(bass2jax.partition_id_tensor())
        outs = bass2jax._bass_exec_p.bind(
            *operands,
            out_avals=tuple(out_avals),
            in_names=tuple(all_names),
            out_names=tuple(out_names),
            lowering_input_output_aliases=(),
            sim_require_finite=True,
            sim_require_nnan=True,
            nc=nc,
        )
        return tuple(outs)

    n_outs = len(out_avals)
    donate = tuple(range(n_params, n_params + n_outs))
    devices = jax.devices()[:8]
    mesh = Mesh(np.asarray(devices), ("core",))
    sharded = jax.jit(
        shard_map(_bass_body, mesh=mesh,
                  in_specs=(PartitionSpec("core"),) * (n_params + n_outs),
                  out_specs=(PartitionSpec("core"),) * n_outs,
                  check_rep=False),
        donate_argnums=donate, keep_unused=True)

    def run(in_maps):
        concat_in = [np.concatenate([np.asarray(m[nm]) for m in in_maps], axis=0)
                     for nm in in_names]
        concat_zero = [np.zeros((8 * s[0], *s[1:]), d) for s, d in zero_shapes]
        out_arrs = sharded(*concat_in, *concat_zero)
        outs = {}
        for i, nm in enumerate(out_names):
            outs[nm] = np.asarray(out_arrs[i]).reshape(8, *zero_shapes[i][0])
        return outs

    return run


def kernel(x, gn_scale, gn_bias, w_qkv, w_out, b_out):
    x = np.asarray(x, dtype=np.float32)
    gn_scale = np.asarray(gn_scale, dtype=np.float32)
    gn_bias = np.asarray(gn_bias, dtype=np.float32)
    w_qkv = np.asarray(w_qkv, dtype=np.float32)
    w_out = np.asarray(w_out, dtype=np.float32)
    b_out = np.asarray(b_out, dtype=np.float32)

    if "run" not in _CACHED:
        _CACHED["run"] = _make_runner()
    in_maps = _shards(x, gn_scale, gn_bias, w_qkv, w_out)
    outs = _CACHED["run"](in_maps)["out"]
    out = np.empty((B, H, W, C), dtype=np.float32)
    for b in range(B):
        s = outs[2 * b] + outs[2 * b + 1] + b_out[None, :]
        out[b] = s.reshape(H, W, C)
    return out


if __name__ == "__main__":
    rng = np.random.default_rng(0)
    x = rng.standard_normal((B, H, W, C)).astype(np.float32)
    out = kernel(x, np.ones(C, np.float32), np.zeros(C, np.float32),
                 (rng.standard_normal((C, 3 * C)) * C ** -0.5).astype(np.float32),
                 (rng.standard_normal((C, C)) * C ** -0.5).astype(np.float32),
                 np.zeros(C, np.float32))
    print(out.shape, out.dtype)
